# revision 1
# baseline (speedup 1.0000x reference)
"""Trainium2 Bass kernel for 16-head MHA with RoPE (B=1, S=4096, D=1024).

Sharding: tensor-parallel over heads — 2 heads per core on 8 cores.
Per-core pipeline (all matmuls bf16, fp32 PSUM accumulation):
  1. Load hidden transposed [d, s] (host-prepared bf16) + weight slices.
  2. Projections: q_T/k_T/v_T [c=128, s=4096] with weight chunks stationary.
  3. RoPE on q_T/k_T in fp32 via partition-swap trick (channels permuted
     host-side to [evens | odds] per head so rotation pairs sit 32 apart).
  4. v_T -> DMA-transpose -> v_nat [s, c]; scaled by f[k]=exp(mask_add[k])
     (exact mask handling folded into V and the denominator vector).
  5. Attention per q-tile of 512: scores computed TRANSPOSED S_T[k, q] so
     softmax needs no vector-engine reductions: exp on ScalarE
     (scale=1/8 folded in), denominator = f^T @ P_T on the PE,
     ctx_T accumulated over 32 k-chunks in PSUM (2 heads col-tiled).
  6. Reciprocal of denominators on DVE, broadcast across partitions via a
     rank-1 PE matmul, single fused normalize+cast to bf16.
  7. Out-projection with ctx_T stationary; fp32 partial written to DRAM.
Host sums the 8 partials.
"""

import functools

import numpy as np
import ml_dtypes

import concourse.bass as bass
import concourse.tile as tile
import concourse.mybir as mybir
from concourse.bass_utils import run_bass_kernel_spmd

BF16 = mybir.dt.bfloat16
F32 = mybir.dt.float32
F32R = mybir.dt.float32r
bf16 = ml_dtypes.bfloat16

S = 4096      # sequence length
D = 1024      # model dim
HD = 64       # head dim
C = 128       # channels per core (2 heads)
NDC = 8       # contraction chunks of 128 over D
NKC = 32      # key chunks of 128 over S
NQT = 8       # query tiles of 512
QT = 512
GRP = 3       # k-chunks per exp group (3 chunks -> 6 PSUM banks? no: 2 banks/chunk)


_NO_SPLIT = (
    mybir.InstEventSemaphore,
    mybir.InstUnconditionalBranch,
)


def _split_multi_waits(nc: bass.Bass) -> None:
    """Hoist extra sem waits onto standalone EventSemaphore carriers.

    This walrus build only supports one sync-wait command per engine
    instruction ("Too many sync wait commands" in setupSyncWait), so any
    instruction Tile scheduled with >1 wait gets all but its last wait moved
    to dedicated InstEventSemaphore instructions placed immediately before it
    in the same engine stream (sequencer blocks on them in program order —
    semantically identical).
    """
    n = 0
    for fn in nc.m.functions:
        for blk in fn.blocks:
            out = []
            for inst in blk.instructions:
                si = inst.sync_info
                if (
                    si is not None
                    and si.on_wait
                    and len(si.on_wait) > 1
                    and not isinstance(inst, _NO_SPLIT)
                    and inst.engine != mybir.EngineType.Unassigned
                ):
                    waits = list(si.on_wait)
                    for w in waits[:-1]:
                        ev = mybir.InstEventSemaphore(name=f"ant_waitsplit_{n}")
                        n += 1
                        ev.engine = inst.engine
                        ev.sync_info = mybir.SyncInfo(on_wait=[w], on_update=[])
                        nc.register_instruction(ev)
                        out.append(ev)
                    si.on_wait = [waits[-1]]
                    inst.sync_info = si
                out.append(inst)
            blk.instructions[:] = out


def build_program() -> bass.Bass:
    nc = bass.Bass()
    hidT_d = nc.declare_dram_parameter("hidT", [D, S], BF16, isOutput=False)
    wq_d = nc.declare_dram_parameter("wq", [128, D], BF16, isOutput=False)
    wk_d = nc.declare_dram_parameter("wk", [128, D], BF16, isOutput=False)
    wv_d = nc.declare_dram_parameter("wv", [128, D], BF16, isOutput=False)
    wo_d = nc.declare_dram_parameter("wo", [128, D], BF16, isOutput=False)
    cos_d = nc.declare_dram_parameter("cosf", [128, S], F32, isOutput=False)
    sin_d = nc.declare_dram_parameter("sinf", [128, S], F32, isOutput=False)
    mask_d = nc.declare_dram_parameter("maskadd", [128, NKC], F32, isOutput=False)
    ones_d = nc.declare_dram_parameter("ones64", [33, 64], F32, isOutput=False)
    out_d = nc.declare_dram_parameter("outp", [S, D], F32, isOutput=True)

    Exp = mybir.ActivationFunctionType.Exp
    mult = mybir.AluOpType.mult
    add = mybir.AluOpType.add

    with tile.TileContext(nc) as tc:
        with (
            tc.tile_pool(name="const", bufs=1) as const,
            tc.tile_pool(name="ppool", bufs=3) as ppool,
        ):
            # ---- persistent SBUF tiles -------------------------------------
            wq_sb = const.tile([128, D], BF16, tag="wq")
            wk_sb = const.tile([128, D], BF16, tag="wk")
            wv_sb = const.tile([128, D], BF16, tag="wv")
            wo_sb = const.tile([128, D], BF16, tag="wo")
            mask_sb = const.tile([128, NKC], F32, tag="mask")
            f_f32 = const.tile([128, NKC], F32, tag="ff32")
            f_bf = const.tile([128, NKC], BF16, tag="fbf")
            ones64 = const.tile([33, 64], F32, tag="ones")
            qT_bf = const.tile([128, S], BF16, tag="qTbf")
            kT_bf = const.tile([128, S], BF16, tag="kTbf")
            v_nat = const.tile([128, S], BF16, tag="vnat")
            ctxn = const.tile([128, S], BF16, tag="ctxn")

            nc.sync.dma_start(out=wq_sb[:], in_=wq_d[:])
            nc.sync.dma_start(out=wk_sb[:], in_=wk_d[:])
            nc.sync.dma_start(out=wv_sb[:], in_=wv_d[:])
            nc.sync.dma_start(out=wo_sb[:], in_=wo_d[:])
            nc.sync.dma_start(out=mask_sb[:], in_=mask_d[:])
            tscratch = const.tile([1, 8], F32, tag="tscratch")
            nc.sync.dma_start(out=ones64[:], in_=ones_d[:])
            # f[k] = exp(mask_add[k]) — also warms the ACT exp table early
            nc.scalar.activation(f_f32[:], mask_sb[:], Exp)
            nc.vector.tensor_copy(f_bf[:], f_f32[:])

            # ---- phase 1: load hidT + projections + rope -------------------
            with (
                tc.tile_pool(name="hid", bufs=1) as hid,
                tc.tile_pool(name="projps", bufs=1, space="PSUM") as projps,
            ):
                hidT_sb = hid.tile([128, NDC * S], BF16, tag="hidT")
                for dc in range(NDC):
                    nc.sync.dma_start(
                        out=hidT_sb[:, dc * S : (dc + 1) * S],
                        in_=hidT_d[dc * 128 : (dc + 1) * 128, :],
                    )
                qT_f32 = hid.tile([128, S], F32, tag="qTf")
                kT_f32 = hid.tile([128, S], F32, tag="kTf")
                vT_bf = hid.tile([128, S], BF16, tag="vTbf")

                def project(w_sb, dst, dst_dtype_cast_only):
                    ps = [projps.tile([128, QT], F32, name=f"pj{st}", tag=f"pj{st}") for st in range(8)]
                    for dc in range(NDC):
                        for st in range(8):
                            nc.tensor.matmul(
                                ps[st][:],
                                lhsT=w_sb[:, dc * 128 : (dc + 1) * 128],
                                rhs=hidT_sb[:, dc * S + st * QT : dc * S + (st + 1) * QT],
                                start=(dc == 0),
                                stop=(dc == NDC - 1),
                            )
                    for st in range(8):
                        nc.vector.tensor_copy(dst[:, st * QT : (st + 1) * QT], ps[st][:])

                project(wq_sb, qT_f32, False)
                project(wk_sb, kT_f32, False)
                project(wv_sb, vT_bf, True)

                # rope streamed in s-segments to bound SBUF: channel rows per
                # head h: [h*64, h*64+32) = even channels ("a"),
                # [h*64+32, h*64+64) = odd ("b");
                # out = x * cos_full + swap(x) * sin_signed
                SEG = S // 2
                with tc.tile_pool(name="ropep", bufs=2) as ropep:
                    for seg in range(2):
                        sc = slice(seg * SEG, (seg + 1) * SEG)
                        cos_sb = ropep.tile([128, SEG], F32, tag="cs")
                        sin_sb = ropep.tile([128, SEG], F32, tag="sn")
                        nc.sync.dma_start(out=cos_sb[:], in_=cos_d[:, sc])
                        nc.sync.dma_start(out=sin_sb[:], in_=sin_d[:, sc])
                        # touch ops absorb the DMA waits on DVE so the rope
                        # tensor_tensor ops stay within the 1-wait TT limit
                        nc.vector.tensor_copy(tscratch[0:1, 0:1], cos_sb[0:1, 0:1])
                        nc.vector.tensor_copy(tscratch[0:1, 1:2], sin_sb[0:1, 0:1])
                        for x_f32, out_bf in ((qT_f32, qT_bf), (kT_f32, kT_bf)):
                            qsw = ropep.tile([128, SEG], F32, tag="qsw", bufs=1)
                            for h in range(2):
                                a = slice(h * 64, h * 64 + 32)
                                b = slice(h * 64 + 32, h * 64 + 64)
                                nc.vector.tensor_copy(qsw[a, :], x_f32[b, sc])
                                nc.vector.tensor_copy(qsw[b, :], x_f32[a, sc])
                            nc.vector.tensor_tensor(
                                x_f32[:, sc], x_f32[:, sc], cos_sb[:], op=mult
                            )
                            nc.vector.tensor_tensor(qsw[:], qsw[:], sin_sb[:], op=mult)
                            nc.vector.tensor_tensor(
                                out_bf[:, sc], x_f32[:, sc], qsw[:], op=add
                            )

                # v_T [c, s] -> v_nat [s, c] stored as 32 chunks [128, 128]
                nc.sync.dma_start_transpose(
                    out=v_nat[:].rearrange("p (kc c) -> p kc c", kc=NKC),
                    in_=vT_bf[:],
                )
                # fold mask factor f[k] into V rows (and later the denominator)
                for kc in range(NKC):
                    nc.vector.tensor_scalar(
                        v_nat[:, kc * 128 : (kc + 1) * 128],
                        v_nat[:, kc * 128 : (kc + 1) * 128],
                        f_f32[:, kc : kc + 1],
                        None,
                        op0=mult,
                    )

            # ---- phase 2: attention ---------------------------------------
            with (
                tc.tile_pool(name="sgps", bufs=2, space="PSUM") as sgps,
                tc.tile_pool(name="ctxps", bufs=1, space="PSUM") as ctxps,
                tc.tile_pool(name="denps", bufs=1, space="PSUM") as denps,
                tc.tile_pool(name="rpool", bufs=2) as rpool,
            ):
                for qt in range(NQT):
                    qc = slice(qt * QT, (qt + 1) * QT)
                    ctx_ps = ctxps.tile([128, QT], F32, tag="ctx")
                    den_ps = denps.tile([128, QT], F32, tag="den")
                    # 64 (chunk, head) score tiles per q-tile, processed in
                    # groups of GRP PSUM banks (double-buffered: 2*GRP banks)
                    slots = [(c, h) for c in range(NKC) for h in range(2)]
                    for g0 in range(0, len(slots), GRP):
                        grp = slots[g0 : g0 + GRP]
                        nb = len(grp)
                        sg = sgps.tile([128, GRP * QT], F32, tag="sg")
                        Pt = ppool.tile([128, GRP * QT], BF16, tag="pt")
                        for i, (c, h) in enumerate(grp):
                            hr = slice(h * 64, (h + 1) * 64)
                            nc.tensor.matmul(
                                sg[:, i * QT : (i + 1) * QT],
                                lhsT=kT_bf[hr, c * 128 : (c + 1) * 128],
                                rhs=qT_bf[hr, qc],
                                start=True,
                                stop=True,
                            )
                        nc.scalar.activation(
                            Pt[:, : nb * QT], sg[:, : nb * QT], Exp, scale=0.125
                        )
                        for i, (c, h) in enumerate(grp):
                            Ps = Pt[:, i * QT : (i + 1) * QT]
                            vcol = c * 128 + h * 64
                            nc.tensor.matmul(
                                ctx_ps[h * 64 : (h + 1) * 64, :],
                                lhsT=v_nat[:, vcol : vcol + 64],
                                rhs=Ps,
                                start=(c == 0),
                                stop=(c == NKC - 1),
                            )
                            nc.tensor.matmul(
                                den_ps[32 * h : 32 * h + 1, :],
                                lhsT=f_bf[:, c : c + 1],
                                rhs=Ps,
                                start=(c == 0),
                                stop=(c == NKC - 1),
                            )
                    recip = rpool.tile([33, QT], F32, tag="recip")
                    # touches: absorb the PE wait (den_ps) and the slot-reuse
                    # WAR waits (recip) ahead of the wait-slot-limited
                    # Reciprocal instructions
                    nc.vector.tensor_copy(tscratch[0:1, 2:3], den_ps[0:1, 0:1])
                    nc.vector.tensor_copy(recip[0:1, 0:1], tscratch[0:1, 2:3])
                    nc.vector.reciprocal(recip[0:1, :], den_ps[0:1, :])
                    nc.vector.reciprocal(recip[32:33, :], den_ps[32:33, :])
                    # broadcast recip across 64 partitions per head via PE
                    nc.tensor.matmul(
                        den_ps[0:64, :],
                        lhsT=ones64[0:1, :],
                        rhs=recip[0:1, :],
                        start=True,
                        stop=True,
                    )
                    nc.tensor.matmul(
                        den_ps[64:128, :],
                        lhsT=ones64[32:33, :],
                        rhs=recip[32:33, :],
                        start=True,
                        stop=True,
                    )
                    recb_sb = rpool.tile([128, QT], F32, tag="recb")
                    nc.vector.tensor_copy(recb_sb[:], den_ps[:])
                    nc.vector.tensor_tensor(
                        ctxn[:, qc], ctx_ps[:], recb_sb[:], op=mult
                    )

            # ---- phase 3: output projection -------------------------------
            with (
                tc.tile_pool(name="ops", bufs=3, space="PSUM") as ops_pool,
                tc.tile_pool(name="outsb", bufs=3) as outsb_pool,
            ):
                for i in range(32):
                    ops_ = ops_pool.tile([128, D], F32, tag="ops")
                    for j in range(2):
                        nc.tensor.matmul(
                            ops_[:, j * QT : (j + 1) * QT],
                            lhsT=ctxn[:, i * 128 : (i + 1) * 128],
                            rhs=wo_sb[:, j * QT : (j + 1) * QT],
                            start=True,
                            stop=True,
                        )
                    osb = outsb_pool.tile([128, D], F32, tag="osb")
                    nc.vector.tensor_copy(osb[:], ops_[:])
                    nc.sync.dma_start(
                        out=out_d[i * 128 : (i + 1) * 128, :], in_=osb[:]
                    )

    _split_multi_waits(nc)
    return nc


@functools.cache
def _cached_program() -> bass.Bass:
    return build_program()


def _prep_inputs(hidden_states, freqs_cis, attention_mask, wq, wk, wv, wo):
    hid = np.asarray(hidden_states, np.float32).reshape(S, D)
    hidT = np.ascontiguousarray(hid.T).astype(bf16)

    # within-head channel permutation: evens then odds (rope pairs 32 apart)
    perm1 = np.concatenate([np.arange(0, HD, 2), np.arange(1, HD, 2)])
    perm = np.concatenate([perm1, perm1 + HD])  # for the 2 heads of a core

    fc = np.asarray(freqs_cis, np.float32)
    cosT = np.ascontiguousarray(fc[:, :, 0].T)  # [32, S]
    sinT = np.ascontiguousarray(fc[:, :, 1].T)
    cosf = np.concatenate([cosT, cosT, cosT, cosT], 0).astype(np.float32)
    sinf = np.concatenate([-sinT, sinT, -sinT, sinT], 0).astype(np.float32)

    mask_add = (1.0 - np.asarray(attention_mask, np.float32).reshape(S)) * -10000.0
    maskadd = np.ascontiguousarray(mask_add.reshape(NKC, 128).T).astype(np.float32)

    def wlayout(w):  # [1024, 128] -> [128 partitions, chunk-major 1024]
        w = np.ascontiguousarray(w)
        return np.ascontiguousarray(
            w.reshape(NDC, 128, 128).transpose(1, 0, 2).reshape(128, D)
        ).astype(bf16)

    in_maps = []
    for core in range(8):
        cols = slice(core * 128, (core + 1) * 128)
        in_maps.append(
            {
                "hidT": hidT,
                "wq": wlayout(np.asarray(wq, np.float32)[:, cols][:, perm]),
                "wk": wlayout(np.asarray(wk, np.float32)[:, cols][:, perm]),
                "wv": wlayout(np.asarray(wv, np.float32)[:, cols]),
                "wo": np.ascontiguousarray(np.asarray(wo, np.float32)[cols, :]).astype(bf16),
                "cosf": cosf,
                "sinf": sinf,
                "maskadd": maskadd,
                "ones64": np.ones((33, 64), np.float32),
            }
        )
    return in_maps


def run_sharded(in_maps, **kwargs):
    nc = _cached_program()
    return run_bass_kernel_spmd(nc, in_maps, list(range(8)), **kwargs)


def kernel(hidden_states, freqs_cis, attention_mask, wq, wk, wv, wo):
    in_maps = _prep_inputs(
        hidden_states, freqs_cis, attention_mask, wq, wk, wv, wo
    )
    res = run_sharded(in_maps).results
    out = np.zeros((S, D), np.float32)
    for r in res:
        out += np.asarray(r["outp"], np.float32)
    return out.reshape(1, S, D)


if __name__ == "__main__":
    import reference

    inputs = reference.setup_inputs()
    inputs = {k: np.asarray(v) for k, v in inputs.items()}
    expected = np.asarray(reference.reference(**inputs))
    actual = kernel(**inputs)
    err = np.abs(actual - expected).max() / np.abs(expected).max()
    print("Relative error:", err)



# revision 4
# speedup vs baseline: 1.5178x; 1.5178x over previous
"""Trainium2 Bass kernel for 16-head MHA with RoPE (B=1, S=4096, D=1024).

Sharding: tensor-parallel over heads — 2 heads per core on 8 cores.
Per-core pipeline (all matmuls bf16, fp32 PSUM accumulation):
  1. Load hidden transposed [d, s] (host-prepared bf16) + weight slices.
  2. Projections: k_T/v_T/q_T [c=128, s=4096] with weight chunks stationary.
  3. RoPE on q_T/k_T in fp32 via partition-swap trick (channels permuted
     host-side to [evens | odds] per head so rotation pairs sit 32 apart).
  4. v_T -> DMA-transpose -> v_nat [s, c] -> reshaped into v_ext with a
     ones-column appended per head: per-chunk blocks [v_h0(64)|1|v_h1(64)|1].
  5. Attention per q-tile of 512, per k-chunk of 128 keys:
     - scores transposed S_T[k, q], the two heads row-split on the PE
       array (tile_position (0,0)/(64,0)) into one [128,1024] PSUM pair;
     - one exp on ScalarE per chunk (scale=1/8, mask folded in as the
       per-partition bias vector);
     - ctx matmuls with the 65-wide v_ext lhsT: row 64 of each head's
       ctx PSUM bank accumulates the softmax denominator for free.
  6. Finalize per q-tile (overlapped into the next q-tile via
     double-buffered ctx banks): copy den rows, one [2,512] reciprocal,
     fp32 PE broadcast matmul to all 128 partitions, fused
     normalize+cast to bf16.
  7. Out-projection with ctx_T stationary; fp32 partial written to DRAM.
Host sums the 8 partials.
"""

import functools

import numpy as np
import ml_dtypes

import concourse.bass as bass
import concourse.tile as tile
import concourse.mybir as mybir
from concourse.bass_utils import run_bass_kernel_spmd

BF16 = mybir.dt.bfloat16
F32 = mybir.dt.float32
bf16 = ml_dtypes.bfloat16

S = 4096      # sequence length
D = 1024      # model dim
HD = 64       # head dim
C = 128       # channels per core (2 heads)
NDC = 8       # contraction chunks of 128 over D
NKC = 32      # key chunks of 128 over S
NQT = 8       # query tiles of 512
QT = 512
VW = 2 * (HD + 1)  # v_ext block width per chunk: [v_h0(64)|1|v_h1(64)|1]


_NO_SPLIT = (
    mybir.InstEventSemaphore,
    mybir.InstUnconditionalBranch,
)


def _split_multi_waits(nc: bass.Bass) -> None:
    """Hoist extra sem waits onto standalone EventSemaphore carriers.

    This walrus build only supports one sync-wait command per engine
    instruction ("Too many sync wait commands" in setupSyncWait), so any
    instruction Tile scheduled with >1 wait gets all but its last wait moved
    to dedicated InstEventSemaphore instructions placed immediately before it
    in the same engine stream (sequencer blocks on them in program order —
    semantically identical).
    """
    n = 0
    for fn in nc.m.functions:
        for blk in fn.blocks:
            out = []
            for inst in blk.instructions:
                si = inst.sync_info
                if (
                    si is not None
                    and si.on_wait
                    and len(si.on_wait) > 1
                    and not isinstance(inst, _NO_SPLIT)
                    and inst.engine != mybir.EngineType.Unassigned
                ):
                    waits = list(si.on_wait)
                    for w in waits[:-1]:
                        ev = mybir.InstEventSemaphore(name=f"ant_waitsplit_{n}")
                        n += 1
                        ev.engine = inst.engine
                        ev.sync_info = mybir.SyncInfo(on_wait=[w], on_update=[])
                        nc.register_instruction(ev)
                        out.append(ev)
                    si.on_wait = [waits[-1]]
                    inst.sync_info = si
                out.append(inst)
            blk.instructions[:] = out


def build_program() -> bass.Bass:
    nc = bass.Bass()
    hidT_d = nc.declare_dram_parameter("hidT", [D, S], BF16, isOutput=False)
    wq_d = nc.declare_dram_parameter("wq", [128, D], BF16, isOutput=False)
    wk_d = nc.declare_dram_parameter("wk", [128, D], BF16, isOutput=False)
    wv_d = nc.declare_dram_parameter("wv", [128, D], BF16, isOutput=False)
    wo_d = nc.declare_dram_parameter("wo", [128, D], BF16, isOutput=False)
    cos_d = nc.declare_dram_parameter("cosf", [128, S], F32, isOutput=False)
    sin_d = nc.declare_dram_parameter("sinf", [128, S], F32, isOutput=False)
    mask_d = nc.declare_dram_parameter("maskadd", [128, NKC], F32, isOutput=False)
    sel_d = nc.declare_dram_parameter("sel2", [33, 128], F32, isOutput=False)
    out_d = nc.declare_dram_parameter("outp", [S, D], F32, isOutput=True)

    Exp = mybir.ActivationFunctionType.Exp
    mult = mybir.AluOpType.mult
    add = mybir.AluOpType.add

    with tile.TileContext(nc) as tc:
        with (
            tc.tile_pool(name="const", bufs=1) as const,
            tc.tile_pool(name="ppool", bufs=3) as ppool,
        ):
            # ---- persistent SBUF tiles -------------------------------------
            wq_sb = const.tile([128, D], BF16, tag="wq")
            wk_sb = const.tile([128, D], BF16, tag="wk")
            wv_sb = const.tile([128, D], BF16, tag="wv")
            wo_sb = const.tile([128, D], BF16, tag="wo")
            mask_sb = const.tile([128, NKC], F32, tag="mask")
            sel_sb = const.tile([33, 128], F32, tag="sel")
            dd_sb = const.tile([33, QT], F32, tag="dd")
            rr_sb = const.tile([33, QT], F32, tag="rr")
            recb_sb = const.tile([128, QT], F32, tag="recb")
            qT_bf = const.tile([128, S], BF16, tag="qTbf")
            kT_bf = const.tile([128, S], BF16, tag="kTbf")
            v_ext = const.tile([128, NKC * VW], BF16, tag="vext")
            ctxn = const.tile([128, S], BF16, tag="ctxn")
            tscratch = const.tile([1, 8], F32, tag="tscratch")

            nc.sync.dma_start(out=wk_sb[:], in_=wk_d[:])
            nc.sync.dma_start(out=wv_sb[:], in_=wv_d[:])
            nc.sync.dma_start(out=wq_sb[:], in_=wq_d[:])
            nc.sync.dma_start(out=mask_sb[:], in_=mask_d[:])
            nc.sync.dma_start(out=sel_sb[:], in_=sel_d[:])
            nc.sync.dma_start(out=wo_sb[:], in_=wo_d[:])
            # ones columns of v_ext (cols HD and 2*HD+1 of each chunk block)
            nc.vector.memset(v_ext[:], 1.0)
            # rows 1..31 of dd stay 1.0 so reciprocal is finite there
            nc.vector.memset(dd_sb[:], 1.0)

            # ---- phase 1: load hidT + projections + rope -------------------
            with (
                tc.tile_pool(name="hid", bufs=1) as hid,
                tc.tile_pool(name="projps", bufs=1, space="PSUM") as projps,
            ):
                hidT_sb = hid.tile([128, NDC * S], BF16, tag="hidT")
                for dc in range(NDC):
                    nc.sync.dma_start(
                        out=hidT_sb[:, dc * S : (dc + 1) * S],
                        in_=hidT_d[dc * 128 : (dc + 1) * 128, :],
                    )
                qT_f32 = hid.tile([128, S], F32, tag="qTf")
                kT_f32 = hid.tile([128, S], F32, tag="kTf")
                vT_bf = hid.tile([128, S], BF16, tag="vTbf")
                v_nat = hid.tile([128, S], BF16, tag="vnat")

                def project(w_sb, dst):
                    ps = [projps.tile([128, QT], F32, name=f"pj{st}", tag=f"pj{st}") for st in range(8)]
                    for dc in range(NDC):
                        for st in range(8):
                            nc.tensor.matmul(
                                ps[st][:],
                                lhsT=w_sb[:, dc * 128 : (dc + 1) * 128],
                                rhs=hidT_sb[:, dc * S + st * QT : dc * S + (st + 1) * QT],
                                start=(dc == 0),
                                stop=(dc == NDC - 1),
                            )
                    for st in range(8):
                        nc.vector.tensor_copy(dst[:, st * QT : (st + 1) * QT], ps[st][:])

                project(wk_sb, kT_f32)
                project(wv_sb, vT_bf)
                project(wq_sb, qT_f32)

                # v_T [c, s] -> v_nat [s, c] stored as 32 chunks [128, 128],
                # then spread into v_ext blocks leaving the ones columns.
                nc.sync.dma_start_transpose(
                    out=v_nat[:].rearrange("p (kc c) -> p kc c", kc=NKC),
                    in_=vT_bf[:],
                )
                nc.vector.tensor_copy(
                    v_ext[:].rearrange("p (kc h w) -> p kc h w", kc=NKC, h=2)[
                        :, :, :, 0:HD
                    ],
                    v_nat[:].rearrange("p (kc h w) -> p kc h w", kc=NKC, h=2),
                )

                # rope streamed in s-segments to bound SBUF: channel rows per
                # head h: [h*64, h*64+32) = even channels ("a"),
                # [h*64+32, h*64+64) = odd ("b");
                # out = x * cos_full + swap(x) * sin_signed
                SEG = S // 2
                with tc.tile_pool(name="ropep", bufs=2) as ropep:
                    for seg in range(2):
                        sc = slice(seg * SEG, (seg + 1) * SEG)
                        cos_sb = ropep.tile([128, SEG], F32, tag="cs")
                        sin_sb = ropep.tile([128, SEG], F32, tag="sn")
                        nc.sync.dma_start(out=cos_sb[:], in_=cos_d[:, sc])
                        nc.sync.dma_start(out=sin_sb[:], in_=sin_d[:, sc])
                        # touch ops absorb the DMA waits on DVE so the rope
                        # tensor_tensor ops stay within the 1-wait TT limit
                        nc.vector.tensor_copy(tscratch[0:1, 0:1], cos_sb[0:1, 0:1])
                        nc.vector.tensor_copy(tscratch[0:1, 1:2], sin_sb[0:1, 0:1])
                        for x_f32, out_bf in ((kT_f32, kT_bf), (qT_f32, qT_bf)):
                            qsw = ropep.tile([128, SEG], F32, tag="qsw", bufs=1)
                            for h in range(2):
                                a = slice(h * 64, h * 64 + 32)
                                b = slice(h * 64 + 32, h * 64 + 64)
                                nc.vector.tensor_copy(qsw[a, :], x_f32[b, sc])
                                nc.vector.tensor_copy(qsw[b, :], x_f32[a, sc])
                            nc.vector.tensor_tensor(
                                x_f32[:, sc], x_f32[:, sc], cos_sb[:], op=mult
                            )
                            nc.vector.tensor_tensor(qsw[:], qsw[:], sin_sb[:], op=mult)
                            nc.vector.tensor_tensor(
                                out_bf[:, sc], x_f32[:, sc], qsw[:], op=add
                            )

            # ---- phase 2: attention ---------------------------------------
            with (
                tc.tile_pool(name="sgps", bufs=2, space="PSUM") as sgps,
                tc.tile_pool(name="ctxps", bufs=2, space="PSUM") as ctxps,
            ):
                # finalize state carried across qtile boundaries
                pending = None  # (ctxA, ctxB, qc)

                def finalize_head(ctxA, ctxB, qt):
                    # den rows (at partitions 0 and 32) -> reciprocal (on DVE)
                    nc.vector.tensor_copy(dd_sb[0:1, :], ctxA[64:65, :])
                    nc.vector.tensor_copy(dd_sb[32:33, :], ctxB[64:65, :])
                    nc.vector.reciprocal(rr_sb[:], dd_sb[:])

                def finalize_tail(ctxA, ctxB, qc):
                    # broadcast 1/den to all partitions via fp32 PE matmul
                    recb = sgps.tile([128, QT], F32, tag="sg")
                    nc.tensor.matmul(
                        recb[:], lhsT=sel_sb[:], rhs=rr_sb[:], start=True, stop=True
                    )
                    nc.vector.tensor_copy(recb_sb[:], recb[:])
                    nc.vector.tensor_tensor(
                        ctxn[0:64, qc], ctxA[0:64, :], recb_sb[0:64, :], op=mult
                    )
                    nc.vector.tensor_tensor(
                        ctxn[64:128, qc], ctxB[0:64, :], recb_sb[64:128, :], op=mult
                    )

                for qt in range(NQT):
                    qc = slice(qt * QT, (qt + 1) * QT)
                    ctxA = ctxps.tile([128, QT], F32, tag="ctxA")
                    ctxB = ctxps.tile([128, QT], F32, tag="ctxB")
                    for c in range(NKC):
                        if c == 2 and pending is not None:
                            finalize_tail(*pending)
                            pending = None
                        sg = sgps.tile([128, 2 * QT], F32, tag="sg")
                        Pt = ppool.tile([128, 2 * QT], BF16, tag="pt")
                        for h in range(2):
                            hr = slice(h * 64, (h + 1) * 64)
                            nc.tensor.matmul(
                                sg[:, h * QT : (h + 1) * QT],
                                lhsT=kT_bf[hr, c * 128 : (c + 1) * 128],
                                rhs=qT_bf[hr, qc],
                                start=True,
                                stop=True,
                            )
                        nc.scalar.activation(
                            Pt[:], sg[:], Exp, bias=mask_sb[:, c : c + 1], scale=0.125
                        )
                        for h, ctx_ps in ((0, ctxA), (1, ctxB)):
                            nc.tensor.matmul(
                                ctx_ps[0:65, :],
                                lhsT=v_ext[
                                    :, c * VW + h * (HD + 1) : c * VW + (h + 1) * (HD + 1)
                                ],
                                rhs=Pt[:, h * QT : (h + 1) * QT],
                                start=(c == 0),
                                stop=(c == NKC - 1),
                            )
                    finalize_head(ctxA, ctxB, qt)
                    pending = (ctxA, ctxB, qc)
                finalize_tail(*pending)

            # ---- phase 3: output projection -------------------------------
            with (
                tc.tile_pool(name="ops", bufs=3, space="PSUM") as ops_pool,
                tc.tile_pool(name="outsb", bufs=3) as outsb_pool,
            ):
                for i in range(32):
                    ops_ = ops_pool.tile([128, D], F32, tag="ops")
                    for j in range(2):
                        nc.tensor.matmul(
                            ops_[:, j * QT : (j + 1) * QT],
                            lhsT=ctxn[:, i * 128 : (i + 1) * 128],
                            rhs=wo_sb[:, j * QT : (j + 1) * QT],
                            start=True,
                            stop=True,
                        )
                    osb = outsb_pool.tile([128, D], F32, tag="osb")
                    nc.vector.tensor_copy(osb[:], ops_[:])
                    nc.sync.dma_start(
                        out=out_d[i * 128 : (i + 1) * 128, :], in_=osb[:]
                    )

    _split_multi_waits(nc)
    return nc


@functools.cache
def _cached_program() -> bass.Bass:
    return build_program()


def _prep_inputs(hidden_states, freqs_cis, attention_mask, wq, wk, wv, wo):
    hid = np.asarray(hidden_states, np.float32).reshape(S, D)
    hidT = np.ascontiguousarray(hid.T).astype(bf16)

    # within-head channel permutation: evens then odds (rope pairs 32 apart)
    perm1 = np.concatenate([np.arange(0, HD, 2), np.arange(1, HD, 2)])
    perm = np.concatenate([perm1, perm1 + HD])  # for the 2 heads of a core

    fc = np.asarray(freqs_cis, np.float32)
    cosT = np.ascontiguousarray(fc[:, :, 0].T)  # [32, S]
    sinT = np.ascontiguousarray(fc[:, :, 1].T)
    cosf = np.concatenate([cosT, cosT, cosT, cosT], 0).astype(np.float32)
    sinf = np.concatenate([-sinT, sinT, -sinT, sinT], 0).astype(np.float32)

    mask_add = (1.0 - np.asarray(attention_mask, np.float32).reshape(S)) * -10000.0
    maskadd = np.ascontiguousarray(mask_add.reshape(NKC, 128).T).astype(np.float32)

    sel2 = np.zeros((33, 128), np.float32)
    sel2[0, 0:64] = 1.0
    sel2[32, 64:128] = 1.0

    def wlayout(w):  # [1024, 128] -> [128 partitions, chunk-major 1024]
        w = np.ascontiguousarray(w)
        return np.ascontiguousarray(
            w.reshape(NDC, 128, 128).transpose(1, 0, 2).reshape(128, D)
        ).astype(bf16)

    in_maps = []
    for core in range(8):
        cols = slice(core * 128, (core + 1) * 128)
        in_maps.append(
            {
                "hidT": hidT,
                "wq": wlayout(np.asarray(wq, np.float32)[:, cols][:, perm]),
                "wk": wlayout(np.asarray(wk, np.float32)[:, cols][:, perm]),
                "wv": wlayout(np.asarray(wv, np.float32)[:, cols]),
                "wo": np.ascontiguousarray(np.asarray(wo, np.float32)[cols, :]).astype(bf16),
                "cosf": cosf,
                "sinf": sinf,
                "maskadd": maskadd,
                "sel2": sel2,
            }
        )
    return in_maps


def run_sharded(in_maps, **kwargs):
    nc = _cached_program()
    return run_bass_kernel_spmd(nc, in_maps, list(range(8)), **kwargs)


def kernel(hidden_states, freqs_cis, attention_mask, wq, wk, wv, wo):
    in_maps = _prep_inputs(
        hidden_states, freqs_cis, attention_mask, wq, wk, wv, wo
    )
    res = run_sharded(in_maps).results
    out = np.zeros((S, D), np.float32)
    for r in res:
        out += np.asarray(r["outp"], np.float32)
    return out.reshape(1, S, D)


if __name__ == "__main__":
    import reference

    inputs = reference.setup_inputs()
    inputs = {k: np.asarray(v) for k, v in inputs.items()}
    expected = np.asarray(reference.reference(**inputs))
    actual = kernel(**inputs)
    err = np.abs(actual - expected).max() / np.abs(expected).max()
    print("Relative error:", err)


# revision 5
# speedup vs baseline: 1.6526x; 1.0888x over previous
"""Trainium2 Bass kernel for 16-head MHA with RoPE (B=1, S=4096, D=1024).

Sharding: tensor-parallel over heads — 2 heads per core on 8 cores.
Per-core pipeline (all matmuls bf16, fp32 PSUM accumulation):
  1. Load hidden transposed [d, s] (host-prepared bf16) + weight slices.
  2. Projections: k_T/v_T/q_T [c=128, s=4096] with weight chunks stationary.
  3. RoPE on q_T/k_T in fp32 via partition-swap trick (channels permuted
     host-side to [evens | odds] per head so rotation pairs sit 32 apart).
  4. v_T -> DMA-transpose -> v_nat [s, c] -> reshaped into v_ext with a
     ones-column appended per head: per-chunk blocks [v_h0(64)|1|v_h1(64)|1].
  5. Attention per q-tile of 512, per k-chunk of 128 keys:
     - scores transposed S_T[k, q], the two heads row-split on the PE
       array (tile_position (0,0)/(64,0)) into one [128,1024] PSUM pair;
     - one exp on ScalarE per chunk (scale=1/8, mask folded in as the
       per-partition bias vector);
     - ctx matmuls with the 65-wide v_ext lhsT: row 64 of each head's
       ctx PSUM bank accumulates the softmax denominator for free.
  6. Finalize per q-tile (overlapped into the next q-tile via
     double-buffered ctx banks): copy den rows, one [2,512] reciprocal,
     fp32 PE broadcast matmul to all 128 partitions, fused
     normalize+cast to bf16.
  7. Out-projection with ctx_T stationary; fp32 partial written to DRAM.
Host sums the 8 partials.
"""

import functools

import numpy as np
import ml_dtypes

import concourse.bass as bass
import concourse.tile as tile
import concourse.mybir as mybir
from concourse.bass_utils import run_bass_kernel_spmd

BF16 = mybir.dt.bfloat16
F32 = mybir.dt.float32
bf16 = ml_dtypes.bfloat16

S = 4096      # sequence length
D = 1024      # model dim
HD = 64       # head dim
C = 128       # channels per core (2 heads)
NDC = 8       # contraction chunks of 128 over D
NKC = 32      # key chunks of 128 over S
NQT = 8       # query tiles of 512
QT = 512
VW = 2 * (HD + 1)  # v_ext block width per chunk: [v_h0(64)|1|v_h1(64)|1]


_NO_SPLIT = (
    mybir.InstEventSemaphore,
    mybir.InstUnconditionalBranch,
)


def _split_multi_waits(nc: bass.Bass) -> None:
    """Hoist extra sem waits onto standalone EventSemaphore carriers.

    This walrus build only supports one sync-wait command per engine
    instruction ("Too many sync wait commands" in setupSyncWait), so any
    instruction Tile scheduled with >1 wait gets all but its last wait moved
    to dedicated InstEventSemaphore instructions placed immediately before it
    in the same engine stream (sequencer blocks on them in program order —
    semantically identical).
    """
    n = 0
    for fn in nc.m.functions:
        for blk in fn.blocks:
            out = []
            for inst in blk.instructions:
                si = inst.sync_info
                if (
                    si is not None
                    and si.on_wait
                    and len(si.on_wait) > 1
                    and not isinstance(inst, _NO_SPLIT)
                    and inst.engine != mybir.EngineType.Unassigned
                ):
                    waits = list(si.on_wait)
                    for w in waits[:-1]:
                        ev = mybir.InstEventSemaphore(name=f"ant_waitsplit_{n}")
                        n += 1
                        ev.engine = inst.engine
                        ev.sync_info = mybir.SyncInfo(on_wait=[w], on_update=[])
                        nc.register_instruction(ev)
                        out.append(ev)
                    si.on_wait = [waits[-1]]
                    inst.sync_info = si
                out.append(inst)
            blk.instructions[:] = out


def build_program() -> bass.Bass:
    nc = bass.Bass()
    hidT_d = nc.declare_dram_parameter("hidT", [D, S], BF16, isOutput=False)
    wq_d = nc.declare_dram_parameter("wq", [128, D], BF16, isOutput=False)
    wk_d = nc.declare_dram_parameter("wk", [128, D], BF16, isOutput=False)
    wv_d = nc.declare_dram_parameter("wv", [128, D], BF16, isOutput=False)
    wo_d = nc.declare_dram_parameter("wo", [128, D], BF16, isOutput=False)
    cos_d = nc.declare_dram_parameter("cosf", [128, S], F32, isOutput=False)
    sin_d = nc.declare_dram_parameter("sinf", [128, S], F32, isOutput=False)
    mask_d = nc.declare_dram_parameter("maskadd", [128, NKC], F32, isOutput=False)
    sel_d = nc.declare_dram_parameter("sel2", [33, 128], F32, isOutput=False)
    out_d = nc.declare_dram_parameter("outp", [S, D], BF16, isOutput=True)

    Exp = mybir.ActivationFunctionType.Exp
    mult = mybir.AluOpType.mult
    add = mybir.AluOpType.add

    with tile.TileContext(nc) as tc:
        with (
            tc.tile_pool(name="const", bufs=1) as const,
            tc.tile_pool(name="ppool", bufs=3) as ppool,
        ):
            # ---- persistent SBUF tiles -------------------------------------
            wq_sb = const.tile([128, D], BF16, tag="wq")
            wk_sb = const.tile([128, D], BF16, tag="wk")
            wv_sb = const.tile([128, D], BF16, tag="wv")
            wo_sb = const.tile([128, D], BF16, tag="wo")
            mask_sb = const.tile([128, NKC], F32, tag="mask")
            sel_sb = const.tile([33, 128], F32, tag="sel")
            dd_sb = const.tile([33, QT], F32, tag="dd")
            rr_sb = const.tile([33, QT], F32, tag="rr")
            recb_sb = const.tile([128, QT], F32, tag="recb")
            qT_bf = const.tile([128, S], BF16, tag="qTbf")
            kT_bf = const.tile([128, S], BF16, tag="kTbf")
            v_ext = const.tile([128, NKC * VW], BF16, tag="vext")
            ctxn = const.tile([128, S], BF16, tag="ctxn")
            tscratch = const.tile([1, 8], F32, tag="tscratch")

            nc.sync.dma_start(out=wk_sb[:], in_=wk_d[:])
            nc.sync.dma_start(out=wv_sb[:], in_=wv_d[:])
            nc.sync.dma_start(out=wq_sb[:], in_=wq_d[:])
            nc.sync.dma_start(out=mask_sb[:], in_=mask_d[:])
            nc.sync.dma_start(out=sel_sb[:], in_=sel_d[:])
            nc.sync.dma_start(out=wo_sb[:], in_=wo_d[:])
            # ones columns of v_ext (cols HD and 2*HD+1 of each chunk block)
            nc.vector.memset(v_ext[:], 1.0)
            # rows 1..31 of dd stay 1.0 so reciprocal is finite there
            nc.vector.memset(dd_sb[:], 1.0)

            # ---- phase 1: load hidT + projections + rope -------------------
            with (
                tc.tile_pool(name="hid", bufs=1) as hid,
                tc.tile_pool(name="projps", bufs=1, space="PSUM") as projps,
            ):
                hidT_sb = hid.tile([128, NDC * S], BF16, tag="hidT")
                for dc in range(NDC):
                    nc.sync.dma_start(
                        out=hidT_sb[:, dc * S : (dc + 1) * S],
                        in_=hidT_d[dc * 128 : (dc + 1) * 128, :],
                    )
                qT_f32 = hid.tile([128, S], F32, tag="qTf")
                kT_f32 = hid.tile([128, S], F32, tag="kTf")
                vT_bf = hid.tile([128, S], BF16, tag="vTbf")
                v_nat = hid.tile([128, S], BF16, tag="vnat")

                def project(w_sb, dst):
                    ps = [projps.tile([128, QT], F32, name=f"pj{st}", tag=f"pj{st}") for st in range(8)]
                    for dc in range(NDC):
                        for st in range(8):
                            nc.tensor.matmul(
                                ps[st][:],
                                lhsT=w_sb[:, dc * 128 : (dc + 1) * 128],
                                rhs=hidT_sb[:, dc * S + st * QT : dc * S + (st + 1) * QT],
                                start=(dc == 0),
                                stop=(dc == NDC - 1),
                            )
                    for st in range(8):
                        nc.vector.tensor_copy(dst[:, st * QT : (st + 1) * QT], ps[st][:])

                project(wk_sb, kT_f32)
                project(wv_sb, vT_bf)
                project(wq_sb, qT_f32)

                # v_T [c, s] -> v_nat [s, c] stored as 32 chunks [128, 128],
                # then spread into v_ext blocks leaving the ones columns.
                nc.sync.dma_start_transpose(
                    out=v_nat[:].rearrange("p (kc c) -> p kc c", kc=NKC),
                    in_=vT_bf[:],
                )
                nc.vector.tensor_copy(
                    v_ext[:].rearrange("p (kc h w) -> p kc h w", kc=NKC, h=2)[
                        :, :, :, 0:HD
                    ],
                    v_nat[:].rearrange("p (kc h w) -> p kc h w", kc=NKC, h=2),
                )

                # rope streamed in s-segments to bound SBUF: channel rows per
                # head h: [h*64, h*64+32) = even channels ("a"),
                # [h*64+32, h*64+64) = odd ("b");
                # out = x * cos_full + swap(x) * sin_signed
                SEG = S // 2
                with tc.tile_pool(name="ropep", bufs=2) as ropep:
                    for seg in range(2):
                        sc = slice(seg * SEG, (seg + 1) * SEG)
                        cos_sb = ropep.tile([128, SEG], F32, tag="cs")
                        sin_sb = ropep.tile([128, SEG], F32, tag="sn")
                        nc.sync.dma_start(out=cos_sb[:], in_=cos_d[:, sc])
                        nc.sync.dma_start(out=sin_sb[:], in_=sin_d[:, sc])
                        # touch ops absorb the DMA waits on DVE so the rope
                        # tensor_tensor ops stay within the 1-wait TT limit
                        nc.vector.tensor_copy(tscratch[0:1, 0:1], cos_sb[0:1, 0:1])
                        nc.vector.tensor_copy(tscratch[0:1, 1:2], sin_sb[0:1, 0:1])
                        for x_f32, out_bf in ((kT_f32, kT_bf), (qT_f32, qT_bf)):
                            qsw = ropep.tile([128, SEG], F32, tag="qsw", bufs=1)
                            for h in range(2):
                                a = slice(h * 64, h * 64 + 32)
                                b = slice(h * 64 + 32, h * 64 + 64)
                                nc.vector.tensor_copy(qsw[a, :], x_f32[b, sc])
                                nc.vector.tensor_copy(qsw[b, :], x_f32[a, sc])
                            nc.vector.tensor_tensor(
                                x_f32[:, sc], x_f32[:, sc], cos_sb[:], op=mult
                            )
                            nc.vector.tensor_tensor(qsw[:], qsw[:], sin_sb[:], op=mult)
                            nc.vector.tensor_tensor(
                                out_bf[:, sc], x_f32[:, sc], qsw[:], op=add
                            )

            # ---- phase 2: attention ---------------------------------------
            with (
                tc.tile_pool(name="sgps", bufs=2, space="PSUM") as sgps,
                tc.tile_pool(name="ctxps", bufs=2, space="PSUM") as ctxps,
            ):
                # finalize state carried across qtile boundaries
                pending = None  # (ctxA, ctxB, qc)

                def finalize_head(ctxA, ctxB, qt):
                    # den rows (at partitions 0 and 32) -> reciprocal (on DVE)
                    nc.vector.tensor_copy(dd_sb[0:1, :], ctxA[64:65, :])
                    nc.vector.tensor_copy(dd_sb[32:33, :], ctxB[64:65, :])
                    nc.vector.reciprocal(rr_sb[:], dd_sb[:])

                def finalize_tail(ctxA, ctxB, qc):
                    # broadcast 1/den to all partitions via fp32 PE matmul
                    recb = sgps.tile([128, QT], F32, tag="sg")
                    nc.tensor.matmul(
                        recb[:], lhsT=sel_sb[:], rhs=rr_sb[:], start=True, stop=True
                    )
                    nc.vector.tensor_copy(recb_sb[:], recb[:])
                    nc.vector.tensor_tensor(
                        ctxn[0:64, qc], ctxA[0:64, :], recb_sb[0:64, :], op=mult
                    )
                    nc.vector.tensor_tensor(
                        ctxn[64:128, qc], ctxB[0:64, :], recb_sb[64:128, :], op=mult
                    )

                for qt in range(NQT):
                    qc = slice(qt * QT, (qt + 1) * QT)
                    ctxA = ctxps.tile([128, QT], F32, tag="ctxA")
                    ctxB = ctxps.tile([128, QT], F32, tag="ctxB")
                    for c in range(NKC):
                        if c == 6 and pending is not None:
                            finalize_tail(*pending)
                            pending = None
                        sg = sgps.tile([128, 2 * QT], F32, tag="sg")
                        Pt = ppool.tile([128, 2 * QT], BF16, tag="pt")
                        for h in range(2):
                            hr = slice(h * 64, (h + 1) * 64)
                            nc.tensor.matmul(
                                sg[:, h * QT : (h + 1) * QT],
                                lhsT=kT_bf[hr, c * 128 : (c + 1) * 128],
                                rhs=qT_bf[hr, qc],
                                start=True,
                                stop=True,
                            )
                        nc.scalar.activation(
                            Pt[:], sg[:], Exp, bias=mask_sb[:, c : c + 1], scale=0.125
                        )
                        for h, ctx_ps in ((0, ctxA), (1, ctxB)):
                            nc.tensor.matmul(
                                ctx_ps[0:65, :],
                                lhsT=v_ext[
                                    :, c * VW + h * (HD + 1) : c * VW + (h + 1) * (HD + 1)
                                ],
                                rhs=Pt[:, h * QT : (h + 1) * QT],
                                start=(c == 0),
                                stop=(c == NKC - 1),
                            )
                    finalize_head(ctxA, ctxB, qt)
                    pending = (ctxA, ctxB, qc)
                finalize_tail(*pending)

            # ---- phase 3: output projection -------------------------------
            with (
                tc.tile_pool(name="ops", bufs=3, space="PSUM") as ops_pool,
                tc.tile_pool(name="outsb", bufs=3) as outsb_pool,
            ):
                for i in range(32):
                    ops_ = ops_pool.tile([128, D], F32, tag="ops")
                    for j in range(2):
                        nc.tensor.matmul(
                            ops_[:, j * QT : (j + 1) * QT],
                            lhsT=ctxn[:, i * 128 : (i + 1) * 128],
                            rhs=wo_sb[:, j * QT : (j + 1) * QT],
                            start=True,
                            stop=True,
                        )
                    osb = outsb_pool.tile([128, D], BF16, tag="osb")
                    nc.vector.tensor_copy(osb[:], ops_[:])
                    nc.sync.dma_start(
                        out=out_d[i * 128 : (i + 1) * 128, :], in_=osb[:]
                    )

    _split_multi_waits(nc)
    return nc


@functools.cache
def _cached_program() -> bass.Bass:
    return build_program()


def _prep_inputs(hidden_states, freqs_cis, attention_mask, wq, wk, wv, wo):
    hid = np.asarray(hidden_states, np.float32).reshape(S, D)
    hidT = np.ascontiguousarray(hid.T).astype(bf16)

    # within-head channel permutation: evens then odds (rope pairs 32 apart)
    perm1 = np.concatenate([np.arange(0, HD, 2), np.arange(1, HD, 2)])
    perm = np.concatenate([perm1, perm1 + HD])  # for the 2 heads of a core

    fc = np.asarray(freqs_cis, np.float32)
    cosT = np.ascontiguousarray(fc[:, :, 0].T)  # [32, S]
    sinT = np.ascontiguousarray(fc[:, :, 1].T)
    cosf = np.concatenate([cosT, cosT, cosT, cosT], 0).astype(np.float32)
    sinf = np.concatenate([-sinT, sinT, -sinT, sinT], 0).astype(np.float32)

    mask_add = (1.0 - np.asarray(attention_mask, np.float32).reshape(S)) * -10000.0
    maskadd = np.ascontiguousarray(mask_add.reshape(NKC, 128).T).astype(np.float32)

    sel2 = np.zeros((33, 128), np.float32)
    sel2[0, 0:64] = 1.0
    sel2[32, 64:128] = 1.0

    def wlayout(w):  # [1024, 128] -> [128 partitions, chunk-major 1024]
        w = np.ascontiguousarray(w)
        return np.ascontiguousarray(
            w.reshape(NDC, 128, 128).transpose(1, 0, 2).reshape(128, D)
        ).astype(bf16)

    in_maps = []
    for core in range(8):
        cols = slice(core * 128, (core + 1) * 128)
        in_maps.append(
            {
                "hidT": hidT,
                "wq": wlayout(np.asarray(wq, np.float32)[:, cols][:, perm]),
                "wk": wlayout(np.asarray(wk, np.float32)[:, cols][:, perm]),
                "wv": wlayout(np.asarray(wv, np.float32)[:, cols]),
                "wo": np.ascontiguousarray(np.asarray(wo, np.float32)[cols, :]).astype(bf16),
                "cosf": cosf,
                "sinf": sinf,
                "maskadd": maskadd,
                "sel2": sel2,
            }
        )
    return in_maps


def run_sharded(in_maps, **kwargs):
    nc = _cached_program()
    return run_bass_kernel_spmd(nc, in_maps, list(range(8)), **kwargs)


def kernel(hidden_states, freqs_cis, attention_mask, wq, wk, wv, wo):
    in_maps = _prep_inputs(
        hidden_states, freqs_cis, attention_mask, wq, wk, wv, wo
    )
    res = run_sharded(in_maps).results
    out = np.zeros((S, D), np.float32)
    for r in res:
        out += np.asarray(r["outp"], np.float32)
    return out.reshape(1, S, D)


if __name__ == "__main__":
    import reference

    inputs = reference.setup_inputs()
    inputs = {k: np.asarray(v) for k, v in inputs.items()}
    expected = np.asarray(reference.reference(**inputs))
    actual = kernel(**inputs)
    err = np.abs(actual - expected).max() / np.abs(expected).max()
    print("Relative error:", err)
